# revision 1
# baseline (speedup 1.0000x reference)
"""Trainium2 Bass kernel for nn_CrossAttention (B=2, Lq=Lkv=2048, E=1024, H=16, D=64).

Head-sharded (2 heads/core), bf16 datapath, with:
  - host-side mask packing: masked-out kv positions are dropped before the
    device sees them (exact; softmax over the surviving set is identical),
    KC=1280 capacity vs 2048 raw.
  - K/V projections share one streamed x tile (half the input DMA).
  - context matmuls contract all 128 k-rows at once; a ones column in the
    stationary makes each also emit the softmax denominator in PSUM.
  - software-pipelined tile loop: scores(kc) issue ahead of ctx(kc-1) so the
    exp (scalar engine) hides under PE work; output-projection and the next
    tile's Q-projection matmuls are interleaved into the attention loop.
  - output partials in bf16; host sums the 8 partials and adds bo.
"""

import sys

if "/opt/trn_rl_repo" not in sys.path:
    sys.path.insert(0, "/opt/trn_rl_repo")

import numpy as np
import ml_dtypes

import concourse.tile as tile
from concourse import bacc, mybir
from concourse.bass_utils import run_bass_kernel_spmd
from concourse.masks import make_identity

F32 = mybir.dt.float32
BF16 = mybir.dt.bfloat16
AF = mybir.ActivationFunctionType
BF = ml_dtypes.bfloat16

N_CORES = 8
B, LQ, LKV, E, H, D = 2, 2048, 2048, 1024, 16, 64
HC = H // N_CORES  # 2 heads per core
JC = HC * D  # 128
T = B * LQ  # 4096
NEC = E // 128  # 8
NOC = E // 128  # 8
NQT = LQ // 512  # 4
NTT = B * NQT  # 8

_NC_CACHE = {}


def build(reps=None, KC=1280):
    key = (reps or 0, KC)
    if key in _NC_CACHE:
        return _NC_CACHE[key]
    NKT = KC // 128  # k chunks per batch
    TKV = B * KC
    NKVT = TKV // 512  # kv projection tiles

    nc = bacc.Bacc("TRN2", target_bir_lowering=False, debug=False, num_devices=N_CORES)

    xqT = nc.dram_tensor("xqT", [E, T], BF16, kind="ExternalInput").ap()
    xkT = nc.dram_tensor("xkT", [E, TKV], BF16, kind="ExternalInput").ap()
    wqT = nc.dram_tensor("wqT", [E, JC], BF16, kind="ExternalInput").ap()
    wkT = nc.dram_tensor("wkT", [E, JC], BF16, kind="ExternalInput").ap()
    wvT = nc.dram_tensor("wvT", [E, JC], BF16, kind="ExternalInput").ap()
    woT = nc.dram_tensor("woT", [JC, E], BF16, kind="ExternalInput").ap()
    bqd = nc.dram_tensor("bq", [JC, 1], F32, kind="ExternalInput").ap()
    bkd = nc.dram_tensor("bk", [JC, 1], F32, kind="ExternalInput").ap()
    bvd = nc.dram_tensor("bv", [JC, 1], F32, kind="ExternalInput").ap()
    mbd = nc.dram_tensor("mb", [B, NKT, 128], F32, kind="ExternalInput").ap()
    outT = nc.dram_tensor("outT", [E, T], BF16, kind="ExternalOutput").ap()

    from contextlib import nullcontext

    with tile.TileContext(nc) as tc, nc.allow_low_precision(reason="bf16 kernel"):
        with tc.For_i(0, reps, 1) if reps else nullcontext():
         with (
             tc.tile_pool(name="const", bufs=1) as const,
             tc.tile_pool(name="big", bufs=1) as big,
         ):
            wq_sb = const.tile([128, NEC, JC], BF16, tag="wq")
            nc.sync.dma_start(out=wq_sb, in_=wqT.rearrange("(ec p) j -> p ec j", p=128))
            wk_sb = const.tile([128, NEC, JC], BF16, tag="wk")
            nc.sync.dma_start(out=wk_sb, in_=wkT.rearrange("(ec p) j -> p ec j", p=128))
            wv_sb = const.tile([128, NEC, JC], BF16, tag="wv")
            nc.sync.dma_start(out=wv_sb, in_=wvT.rearrange("(ec p) j -> p ec j", p=128))
            wo_sb = const.tile([128, NOC, 128], BF16, tag="wo")
            nc.sync.dma_start(out=wo_sb, in_=woT.rearrange("p (oc o) -> p oc o", oc=NOC))
            bq_sb = const.tile([128, 1], F32, tag="bq")
            nc.sync.dma_start(out=bq_sb, in_=bqd)
            bk_sb = const.tile([128, 1], F32, tag="bk")
            nc.sync.dma_start(out=bk_sb, in_=bkd)
            bv_sb = const.tile([128, 1], F32, tag="bv")
            nc.sync.dma_start(out=bv_sb, in_=bvd)
            mb_sb = const.tile([128, B, NKT], F32, tag="mb")
            nc.sync.dma_start(out=mb_sb, in_=mbd.rearrange("b kc p -> p b kc"))
            ident = const.tile([128, 128], BF16, tag="ident")
            make_identity(nc, ident)
            onesP = const.tile([128, 65], BF16, tag="onesP")
            nc.vector.memset(onesP, 1.0)

            kt_sb = big.tile([128, TKV], BF16, tag="kt")
            vt_sb = big.tile([128, TKV], BF16, tag="vt")
            v_sb = big.tile([128, B * NKT, 130], BF16, tag="v")

            # ---- phase KV: K/V projections from one streamed x tile ----
            with (
                tc.tile_pool(name="xkv", bufs=2) as xkv,
                tc.tile_pool(name="kvp", bufs=2, space="PSUM") as kvp,
            ):
                for i in range(NKVT):
                    xt = xkv.tile([128, NEC, 512], BF16, tag="xkv")
                    nc.sync.dma_start(
                        out=xt,
                        in_=xkT[:, i * 512 : (i + 1) * 512].rearrange(
                            "(ec p) t -> p ec t", p=128
                        ),
                    )
                    for wsb, bias, dst in (
                        (wk_sb, bk_sb, kt_sb),
                        (wv_sb, bv_sb, vt_sb),
                    ):
                        pt = kvp.tile([128, 512], F32, tag="kvp")
                        for ec in range(NEC):
                            nc.tensor.matmul(
                                pt, wsb[:, ec, :], xt[:, ec, :],
                                start=(ec == 0), stop=(ec == NEC - 1),
                            )
                        nc.scalar.activation(
                            out=dst[:, i * 512 : (i + 1) * 512],
                            in_=pt, func=AF.Identity, bias=bias, scale=1.0,
                        )

            # ---- phase T: V^T -> v_sb [k, gc, [Vh0|1|1|Vh1]] ----
            with tc.tile_pool(name="tp", bufs=3, space="PSUM") as tp:
                nc.vector.memset(v_sb[:, :, 64:66], 1.0)
                for gc in range(B * NKT):
                    tpt = tp.tile([128, 128], BF16, tag="tp")
                    nc.tensor.transpose(
                        tpt, vt_sb[:, gc * 128 : (gc + 1) * 128], ident
                    )
                    nc.vector.tensor_copy(v_sb[:, gc, 0:64], tpt[:, 0:64])
                    nc.vector.tensor_copy(v_sb[:, gc, 66:130], tpt[:, 64:128])

            # ---- phase QAO: pipelined per-512-token tile ----
            with (
                tc.tile_pool(name="xq", bufs=3) as xqp,
                tc.tile_pool(name="qt", bufs=3) as qtp,
                tc.tile_pool(name="emt", bufs=3) as emtp,
                tc.tile_pool(name="ctx", bufs=2) as ctxp,
                tc.tile_pool(name="cs1", bufs=2) as cs1p,
                tc.tile_pool(name="rr", bufs=2) as rrp,
                tc.tile_pool(name="outsb", bufs=2) as outp,
                tc.tile_pool(name="ps2", bufs=2, space="PSUM") as ps2,
                tc.tile_pool(name="ps1", bufs=1, space="PSUM") as ps1,
            ):
                state = {}

                def emit_dma_xq(tt):
                    xt = xqp.tile([128, NEC, 512], BF16, tag="xq", name=f"xq_{tt}")
                    nc.sync.dma_start(
                        out=xt,
                        in_=xqT[:, tt * 512 : (tt + 1) * 512].rearrange(
                            "(ec p) t -> p ec t", p=128
                        ),
                    )
                    state[("xq", tt)] = xt

                def emit_qproj_mm(tt, ec):
                    if ec == 0:
                        state[("qp", tt)] = ps1.tile(
                            [128, 512], F32, tag="aux", name=f"qp_{tt}"
                        )
                    nc.tensor.matmul(
                        state[("qp", tt)], wq_sb[:, ec, :],
                        state[("xq", tt)][:, ec, :],
                        start=(ec == 0), stop=(ec == NEC - 1),
                    )

                def emit_qproj_act(tt):
                    qt = qtp.tile([128, 512], BF16, tag="qt", name=f"qt_{tt}")
                    nc.scalar.activation(
                        out=qt, in_=state[("qp", tt)],
                        func=AF.Identity, bias=bq_sb, scale=1.0,
                    )
                    state[("qt", tt)] = qt

                def emit_epi0(tt):
                    # head0: denom at cx0 row 64; ctx rows 0-63
                    rr = rrp.tile([65, 512], BF16, tag="rr", name=f"rr_{tt}")
                    state[("rr", tt)] = rr
                    cx0, cx1 = state[("cx", tt)]
                    nc.vector.reciprocal(rr[64:65, :], cx0[64:65, :])
                    nc.vector.reciprocal(rr[0:1, :], cx1[0:1, :])

                def emit_epi1(tt):
                    cx0, _ = state[("cx", tt)]
                    rr = state[("rr", tt)]
                    bt = ps1.tile([128, 512], F32, tag="aux", name=f"bct0_{tt}")
                    nc.tensor.matmul(
                        bt[0:65, :], onesP[64:65, :], rr[64:65, :],
                        start=True, stop=True,
                    )
                    s0 = cs1p.tile([65, 512], BF16, tag="s0", name=f"s0_{tt}")
                    nc.vector.tensor_copy(s0, cx0)
                    ctx = ctxp.tile([128, 512], BF16, tag="ctx", name=f"ctx_{tt}")
                    state[("ctx", tt)] = ctx
                    nc.vector.tensor_mul(ctx[0:64, :], s0[0:64, :], bt[0:64, :])

                def emit_epi2(tt):
                    # head1: denom at cx1 row 0; ctx rows 1-64 -> shift via DMA
                    _, cx1 = state[("cx", tt)]
                    rr = state[("rr", tt)]
                    bt = ps1.tile([128, 512], F32, tag="aux", name=f"bct1_{tt}")
                    nc.tensor.matmul(
                        bt[0:65, :], onesP[0:1, :], rr[0:1, :],
                        start=True, stop=True,
                    )
                    s1 = cs1p.tile([65, 512], BF16, tag="s1", name=f"s1_{tt}")
                    nc.vector.tensor_copy(s1, cx1)
                    cs = cs1p.tile([65, 512], BF16, tag="cs1", name=f"cs1_{tt}")
                    nc.vector.tensor_mul(cs, s1, bt[0:65, :])
                    nc.sync.dma_start(
                        out=state[("ctx", tt)][64:128, :], in_=cs[1:65, :]
                    )

                def emit_omm(tt, oc):
                    if oc == 0:
                        state[("ob", tt)] = outp.tile(
                            [128, NOC, 512], BF16, tag="ob", name=f"ob_{tt}"
                        )
                    op = ps1.tile([128, 512], F32, tag="op", name=f"op_{tt}_{oc}")
                    nc.tensor.matmul(
                        op, wo_sb[:, oc, :], state[("ctx", tt)],
                        start=True, stop=True,
                    )
                    nc.vector.tensor_copy(state[("ob", tt)][:, oc, :], op)

                def emit_outdma(tt):
                    nc.sync.dma_start(
                        out=outT[:, tt * 512 : (tt + 1) * 512].rearrange(
                            "(oc p) t -> p oc t", p=128
                        ),
                        in_=state[("ob", tt)],
                    )
                    del state[("ob", tt)], state[("ctx", tt)]

                emit_dma_xq(0)
                emit_dma_xq(1)
                for ec in range(NEC):
                    emit_qproj_mm(0, ec)
                emit_qproj_act(0)

                for tt in range(NTT):
                    b = tt // NQT
                    if tt + 2 < NTT:
                        emit_dma_xq(tt + 2)
                    cx0 = ps1.tile([65, 512], F32, tag="cx0", name=f"cx0_{tt}")
                    cx1 = ps1.tile([65, 512], F32, tag="cx1", name=f"cx1_{tt}")
                    state[("cx", tt)] = (cx0, cx1)
                    qt = state[("qt", tt)]
                    for kc in range(NKT + 1):
                        if kc < NKT:
                            k0 = (b * NKT + kc) * 128
                            sct = ps2.tile(
                                [128, 2, 512], F32, tag="sct", name=f"sct_{tt}_{kc}"
                            )
                            nc.tensor.matmul(
                                sct[:, 0, :], kt_sb[0:64, k0 : k0 + 128],
                                qt[0:64, :], start=True, stop=True,
                            )
                            nc.tensor.matmul(
                                sct[:, 1, :], kt_sb[64:128, k0 : k0 + 128],
                                qt[64:128, :], start=True, stop=True,
                            )
                            emt = emtp.tile(
                                [128, 2, 512], BF16, tag="emt", name=f"emt_{tt}_{kc}"
                            )
                            nc.scalar.activation(
                                out=emt.rearrange("p a t -> p (a t)"),
                                in_=sct.rearrange("p a t -> p (a t)"),
                                func=AF.Exp,
                                bias=mb_sb[:, b, kc : kc + 1],
                                scale=0.125,
                            )
                            state[("emt", kc)] = emt
                        # interleaved extras: prev tile epilogue+O, next tile Q
                        if kc == 0 and tt > 0:
                            emit_epi1(tt - 1)
                        elif kc == 1 and tt > 0:
                            emit_epi2(tt - 1)
                        elif 2 <= kc < 2 + NOC and tt > 0:
                            emit_omm(tt - 1, kc - 2)
                        if 2 <= kc < 2 + NEC and tt + 1 < NTT:
                            emit_qproj_mm(tt + 1, kc - 2)
                        if kc >= 1:
                            kp = kc - 1
                            gc = b * NKT + kp
                            emp = state[("emt", kp)]
                            st, sp = (kp == 0), (kp == NKT - 1)
                            nc.tensor.matmul(
                                cx0, v_sb[:, gc, 0:65], emp[:, 0, :],
                                start=st, stop=sp,
                            )
                            nc.tensor.matmul(
                                cx1, v_sb[:, gc, 65:130], emp[:, 1, :],
                                start=st, stop=sp,
                            )
                    if tt + 1 < NTT:
                        emit_qproj_act(tt + 1)
                    emit_epi0(tt)
                    if tt > 0:
                        emit_outdma(tt - 1)

                tt = NTT - 1
                emit_epi1(tt)
                emit_epi2(tt)
                for oc in range(NOC):
                    emit_omm(tt, oc)
                emit_outdma(tt)

    nc.compile()
    _NC_CACHE[key] = nc
    return nc


def _pick_kc(mask):
    mx = max(int((mask[b] != 0).sum()) for b in range(B))
    for kc in (1280, 1536, 1792, 2048):
        if mx <= kc:
            return kc
    return 2048


def make_in_maps(query, key_value, mask, Wq, bq, Wk, bk, Wv, bv, Wo, bo, KC=1280):
    NKT = KC // 128
    xqT = np.ascontiguousarray(
        np.asarray(query, np.float32).reshape(T, E).T
    ).astype(BF)
    kvp = np.zeros((B, KC, E), np.float32)
    mbias = np.full((B, KC), -1.0e5, np.float32)
    kv = np.asarray(key_value, np.float32)
    for b in range(B):
        idx = np.nonzero(np.asarray(mask)[b] != 0)[0]
        n = min(len(idx), KC)
        kvp[b, :n] = kv[b][idx[:n]]
        mbias[b, :n] = 0.0
    xkT = np.ascontiguousarray(kvp.reshape(B * KC, E).T).astype(BF)
    mb = mbias.reshape(B, NKT, 128)
    in_maps = []
    for c in range(N_CORES):
        sl = slice(c * JC, (c + 1) * JC)
        in_maps.append(
            {
                "xqT": xqT,
                "xkT": xkT,
                "wqT": np.ascontiguousarray(np.asarray(Wq)[sl, :].T).astype(BF),
                "wkT": np.ascontiguousarray(np.asarray(Wk)[sl, :].T).astype(BF),
                "wvT": np.ascontiguousarray(np.asarray(Wv)[sl, :].T).astype(BF),
                "woT": np.ascontiguousarray(np.asarray(Wo)[:, sl].T).astype(BF),
                "bq": np.asarray(bq)[sl].reshape(JC, 1).astype(np.float32),
                "bk": np.asarray(bk)[sl].reshape(JC, 1).astype(np.float32),
                "bv": np.asarray(bv)[sl].reshape(JC, 1).astype(np.float32),
                "mb": mb,
            }
        )
    return in_maps


def kernel(query, key_value, mask, Wq, bq, Wk, bk, Wv, bv, Wo, bo):
    KC = _pick_kc(np.asarray(mask))
    nc = build(None, KC)
    in_maps = make_in_maps(
        query, key_value, mask, Wq, bq, Wk, bk, Wv, bv, Wo, bo, KC=KC
    )
    res = run_bass_kernel_spmd(nc, in_maps, list(range(N_CORES)))
    acc = np.zeros((E, T), np.float32)
    for c in range(N_CORES):
        acc += np.asarray(res.results[c]["outT"], dtype=np.float32)
    acc += np.asarray(bo, np.float32).reshape(E, 1)
    return np.ascontiguousarray(acc.T).reshape(B, LQ, E).astype(np.float32)



# revision 52
# speedup vs baseline: 1.1714x; 1.1714x over previous
"""Trainium2 Bass kernel for nn_CrossAttention (B=2, Lq=Lkv=2048, E=1024, H=16, D=64).

Head-sharded (2 heads/core), bf16 datapath, with:
  - host-side mask packing: masked-out kv positions are dropped before the
    device sees them (exact; softmax over the surviving set is identical),
    KC chosen per the actual survivor count (1024 for the reference mask).
  - host-side weight/bias images laid out as the exact SBUF tile memory so
    every DMA moves >=1KB contiguous lines (no descriptor RMW penalty).
  - K/V projections share one streamed x tile (half the input DMA); V-chunk
    transposes interleave with the projections; Q-projection of the first
    tile rides the projection-phase PSUM pool so attention starts hot.
  - context matmuls contract all 128 k-rows at once; a ones column in the
    stationary makes each also emit the softmax denominator in PSUM.
  - software-pipelined tile loop: scores(kc) issue ahead of ctx(kc-1) so the
    exp (scalar engine) hides under PE work; output-projection and the next
    tile's Q-projection matmuls are interleaved into the attention loop, with
    the next tile's qt activation emitted mid-loop so it never queues behind
    the current tile's last exps.
  - PSUM evacuation mostly on DVE with the spill-over on ACT; the final
    tile's O-projection borrows the idle score PSUM banks.
  - the timing (reps) loop carries branch-prefetch hints on all engines so
    the back-edge I$-hits instead of stalling on an IRAM fetch.
  - output partials in bf16; host sums the 8 partials and adds bo.
"""

import sys

if "/opt/trn_rl_repo" not in sys.path:
    sys.path.insert(0, "/opt/trn_rl_repo")

import numpy as np
import ml_dtypes

import concourse.tile as tile
from concourse import bacc, mybir
from concourse.bass_utils import run_bass_kernel_spmd
from concourse.masks import make_identity

F32 = mybir.dt.float32
BF16 = mybir.dt.bfloat16
AF = mybir.ActivationFunctionType
BF = ml_dtypes.bfloat16

N_CORES = 8
B, LQ, LKV, E, H, D = 2, 2048, 2048, 1024, 16, 64
HC = H // N_CORES  # 2 heads per core
JC = HC * D  # 128
T = B * LQ  # 4096
NEC = E // 128  # 8
NOC = E // 128  # 8
NQT = LQ // 512  # 4
NTT = B * NQT  # 8

_NC_CACHE = {}


def build(reps=None, KC=1024):
    key = (reps or 0, KC)
    if key in _NC_CACHE:
        return _NC_CACHE[key]
    NKT = KC // 128  # k chunks per batch
    TKV = B * KC
    NKVT = TKV // 512  # kv projection tiles
    NAUX = 3 + B * NKT

    nc = bacc.Bacc("TRN2", target_bir_lowering=False, debug=False, num_devices=N_CORES)

    xqT = nc.dram_tensor("xqT", [E, T], BF16, kind="ExternalInput").ap()
    xkT = nc.dram_tensor("xkT", [E, TKV], BF16, kind="ExternalInput").ap()
    # packed weight images: [wk|wv] and [wq|wo] ride one DMA each
    wkvi = nc.dram_tensor("wkvi", [128, 2 * NEC * JC], BF16, kind="ExternalInput").ap()
    wqoi = nc.dram_tensor("wqoi", [128, 2 * NEC * JC], BF16, kind="ExternalInput").ap()
    auxi = nc.dram_tensor("auxi", [128, NAUX], F32, kind="ExternalInput").ap()
    outT = nc.dram_tensor("outT", [E, T], BF16, kind="ExternalOutput").ap()

    from contextlib import nullcontext

    ET = mybir.EngineType
    hint = (ET.PE, ET.Activation, ET.DVE, ET.SP, ET.Pool)
    with tile.TileContext(nc) as tc, nc.allow_low_precision(reason="bf16 kernel"):
        with (
            tc.For_i(0, reps, 1, hint_engines=hint, staggered_reset=True)
            if reps
            else nullcontext()
        ):
         with (
             tc.tile_pool(name="const", bufs=1) as const,
             tc.tile_pool(name="big", bufs=1) as big,
             tc.tile_pool(name="xq", bufs=3) as xqp,
             tc.tile_pool(name="qt", bufs=3) as qtp,
         ):
            # K/V weights + aux first: K-projection is the critical path at
            # kernel start (SP DMA queue drains in emission order).
            wkv_sb = const.tile([128, 2, NEC, JC], BF16, tag="wkv")
            nc.sync.dma_start(
                out=wkv_sb,
                in_=wkvi.rearrange("p (two ec j) -> p two ec j", two=2, ec=NEC),
            )
            wk_sb = wkv_sb[:, 0]
            wv_sb = wkv_sb[:, 1]
            aux_sb = const.tile([128, NAUX], F32, tag="aux_c")
            nc.sync.dma_start(out=aux_sb, in_=auxi)
            bq_sb = aux_sb[:, 0:1]
            bk_sb = aux_sb[:, 1:2]
            bv_sb = aux_sb[:, 2:3]
            mb_sb = aux_sb[:, 3:].rearrange("p (b kc) -> p b kc", b=B)
            ident = const.tile([128, 128], BF16, tag="ident")
            make_identity(nc, ident)
            onesP = const.tile([128, 65], BF16, tag="onesP")
            nc.vector.memset(onesP, 1.0)

            kt_sb = big.tile([128, TKV], BF16, tag="kt")
            vt_sb = big.tile([128, TKV], BF16, tag="vt")
            v_sb = big.tile([128, B * NKT, 130], BF16, tag="v")

            state = {}

            def emit_dma_xq_pair(pp):
                # one 2MB DMA covers two 512-token tiles (2*pp, 2*pp+1):
                # HW DMA fixed cost is ~1-2us per dma_start, so batch
                xt = xqp.tile([128, NEC, 1024], BF16, tag="xq", name=f"xqp_{pp}")
                nc.sync.dma_start(
                    out=xt,
                    in_=xqT[:, pp * 1024 : (pp + 1) * 1024].rearrange(
                        "(ec p) t -> p ec t", p=128
                    ),
                )
                state[("xq", 2 * pp)] = xt[:, :, 0:512]
                state[("xq", 2 * pp + 1)] = xt[:, :, 512:1024]

            def emit_qproj_mm(tt, ec, pool, tag):
                if ec == 0:
                    state[("qp", tt)] = pool.tile(
                        [128, 512], F32, tag=tag, name=f"qp_{tt}"
                    )
                nc.tensor.matmul(
                    state[("qp", tt)], wq_sb[:, ec, :],
                    state[("xq", tt)][:, ec, :],
                    start=(ec == 0), stop=(ec == NEC - 1),
                )

            def emit_qproj_act(tt):
                qt = qtp.tile([128, 512], BF16, tag="qt", name=f"qt_{tt}")
                nc.scalar.activation(
                    out=qt, in_=state[("qp", tt)],
                    func=AF.Identity, bias=bq_sb, scale=1.0,
                )
                state[("qt", tt)] = qt

            # ---- phase KV: K/V projections from one streamed x tile ----
            with (
                tc.tile_pool(name="xkv", bufs=2) as xkv,
                tc.tile_pool(name="kvp", bufs=2, space="PSUM") as kvp,
                tc.tile_pool(name="tp", bufs=2, space="PSUM") as tp,
            ):
                nc.gpsimd.memset(v_sb[:, :, 64:66], 1.0)
                wqo_sb = const.tile([128, 2, NEC, JC], BF16, tag="wqo")
                wq_sb = wqo_sb[:, 0]
                wo_sb = wqo_sb[:, 1]
                for i in range(NKVT):
                    xt = xkv.tile([128, NEC, 512], BF16, tag="xkv")
                    nc.sync.dma_start(
                        out=xt,
                        in_=xkT[:, i * 512 : (i + 1) * 512].rearrange(
                            "(ec p) t -> p ec t", p=128
                        ),
                    )
                    if i == 1:
                        # batch-0 kt (kv tiles 0-1) is the attention-start
                        # critical path; only then queue wq/wo + the first xq
                        nc.sync.dma_start(
                            out=wqo_sb,
                            in_=wqoi.rearrange(
                                "p (two ec j) -> p two ec j", two=2, ec=NEC
                            ),
                        )
                        emit_dma_xq_pair(0)
                    for wsb, bias, dst in (
                        (wk_sb, bk_sb, kt_sb),
                        (wv_sb, bv_sb, vt_sb),
                    ):
                        pt = kvp.tile([128, 512], F32, tag="kvp")
                        for ec in range(NEC):
                            nc.tensor.matmul(
                                pt, wsb[:, ec, :], xt[:, ec, :],
                                start=(ec == 0), stop=(ec == NEC - 1),
                            )
                        nc.scalar.activation(
                            out=dst[:, i * 512 : (i + 1) * 512],
                            in_=pt, func=AF.Identity, bias=bias, scale=1.0,
                        )
                    # ---- transpose the V chunks of this tile as they land ----
                    for gc in range(i * 4, i * 4 + 4):
                        tpt = tp.tile([128, 128], BF16, tag="tp")
                        nc.tensor.transpose(
                            tpt, vt_sb[:, gc * 128 : (gc + 1) * 128], ident
                        )
                        nc.vector.tensor_copy(v_sb[:, gc, 0:64], tpt[:, 0:64])
                        nc.vector.tensor_copy(v_sb[:, gc, 66:130], tpt[:, 64:128])
                # tile-0 Q projection rides the kvp psum pool so it overlaps
                # the projection phase instead of delaying attention start
                for ec in range(NEC):
                    emit_qproj_mm(0, ec, kvp, "kvp")
                emit_qproj_act(0)

            # ---- phase QAO: pipelined per-512-token tile ----
            with (
                tc.tile_pool(name="emt", bufs=3) as emtp,
                tc.tile_pool(name="ctx", bufs=2) as ctxp,
                tc.tile_pool(name="cs1", bufs=2) as cs1p,
                tc.tile_pool(name="rr", bufs=2) as rrp,
                tc.tile_pool(name="outsb", bufs=2) as outp,
                tc.tile_pool(name="ps2", bufs=2, space="PSUM") as ps2,
                tc.tile_pool(name="ps1", bufs=1, space="PSUM") as ps1,
            ):
                def emit_epi0(tt):
                    # head0: denom at cx0 row 64; ctx rows 0-63
                    rr = rrp.tile([65, 512], BF16, tag="rr", name=f"rr_{tt}")
                    state[("rr", tt)] = rr
                    cx0, cx1 = state[("cx", tt)]
                    nc.vector.reciprocal(rr[64:65, :], cx0[64:65, :])
                    nc.vector.reciprocal(rr[0:1, :], cx1[0:1, :])

                def emit_epi1(tt):
                    cx0, _ = state[("cx", tt)]
                    rr = state[("rr", tt)]
                    bt = ps1.tile([128, 512], F32, tag="aux", name=f"bct0_{tt}")
                    nc.tensor.matmul(
                        bt[0:65, :], onesP[64:65, :], rr[64:65, :],
                        start=True, stop=True,
                    )
                    s0 = cs1p.tile([65, 512], BF16, tag="s0", name=f"s0_{tt}")
                    nc.vector.tensor_copy(s0, cx0)
                    ctx = ctxp.tile([128, 512], BF16, tag="ctx", name=f"ctx_{tt}")
                    state[("ctx", tt)] = ctx
                    nc.vector.tensor_mul(ctx[0:64, :], s0[0:64, :], bt[0:64, :])

                def emit_epi2(tt):
                    # head1: denom at cx1 row 0; ctx rows 1-64 -> shift via DMA
                    _, cx1 = state[("cx", tt)]
                    rr = state[("rr", tt)]
                    bt = ps1.tile([128, 512], F32, tag="aux", name=f"bct1_{tt}")
                    nc.tensor.matmul(
                        bt[0:65, :], onesP[0:1, :], rr[0:1, :],
                        start=True, stop=True,
                    )
                    s1 = cs1p.tile([65, 512], BF16, tag="s1", name=f"s1_{tt}")
                    nc.vector.tensor_copy(s1, cx1)
                    cs = cs1p.tile([65, 512], BF16, tag="cs1", name=f"cs1_{tt}")
                    nc.vector.tensor_mul(cs, s1, bt[0:65, :])
                    nc.sync.dma_start(
                        out=state[("ctx", tt)][64:128, :], in_=cs[1:65, :]
                    )

                def emit_omm(tt, oc, pool, tag, act=False):
                    if oc == 0:
                        if tt % 2 == 0:
                            state[("obp", tt // 2)] = outp.tile(
                                [128, NOC, 1024], BF16, tag="ob",
                                name=f"obp_{tt // 2}",
                            )
                        obp = state[("obp", tt // 2)]
                        h = (tt % 2) * 512
                        state[("ob", tt)] = obp[:, :, h : h + 512]
                    op = pool.tile([128, 512], F32, tag=tag, name=f"op_{tt}_{oc}")
                    nc.tensor.matmul(
                        op, wo_sb[:, oc, :], state[("ctx", tt)],
                        start=True, stop=True,
                    )
                    if act:
                        nc.scalar.copy(state[("ob", tt)][:, oc, :], op)
                    else:
                        nc.vector.tensor_copy(state[("ob", tt)][:, oc, :], op)

                def emit_outdma_pair(p):
                    # one 2MB DMA stores two finished tiles' partials
                    nc.sync.dma_start(
                        out=outT[:, p * 1024 : (p + 1) * 1024].rearrange(
                            "(oc p2) t -> p2 oc t", p2=128
                        ),
                        in_=state[("obp", p)],
                    )
                    del state[("obp", p)]

                for tt in range(NTT):
                    b = tt // NQT
                    if tt % 2 == 0 and tt // 2 + 1 < NTT // 2:
                        emit_dma_xq_pair(tt // 2 + 1)
                    cx0 = ps1.tile([65, 512], F32, tag="cx0", name=f"cx0_{tt}")
                    cx1 = ps1.tile([65, 512], F32, tag="cx1", name=f"cx1_{tt}")
                    state[("cx", tt)] = (cx0, cx1)
                    qt = state[("qt", tt)]
                    for kc in range(NKT + 1):
                        if kc < NKT:
                            k0 = (b * NKT + kc) * 128
                            sct = ps2.tile(
                                [128, 2, 512], F32, tag="sct", name=f"sct_{tt}_{kc}"
                            )
                            nc.tensor.matmul(
                                sct[:, 0, :], kt_sb[0:64, k0 : k0 + 128],
                                qt[0:64, :], start=True, stop=True,
                            )
                            nc.tensor.matmul(
                                sct[:, 1, :], kt_sb[64:128, k0 : k0 + 128],
                                qt[64:128, :], start=True, stop=True,
                            )
                            emt = emtp.tile(
                                [128, 2, 512], BF16, tag="emt", name=f"emt_{tt}_{kc}"
                            )
                            nc.scalar.activation(
                                out=emt.rearrange("p a t -> p (a t)"),
                                in_=sct.rearrange("p a t -> p (a t)"),
                                func=AF.Exp,
                                bias=mb_sb[:, b, kc : kc + 1],
                                scale=0.125,
                            )
                            state[("emt", kc)] = emt
                        # interleaved extras: prev tile epilogue+O, next tile Q
                        if tt > 0:
                            if kc == 0:
                                emit_epi1(tt - 1)
                            elif kc == 1:
                                emit_epi2(tt - 1)
                            else:
                                # spread the NOC omms over slots kc=2..NKT
                                o0 = (kc - 2) * NOC // (NKT - 1)
                                o1 = (kc - 1) * NOC // (NKT - 1)
                                for oc in range(o0, o1):
                                    emit_omm(tt - 1, oc, ps1, "op",
                                             act=(oc >= 6))
                        if 2 <= kc <= 5 and tt + 1 < NTT:
                            # qproj mms packed into slots kc=2..5 and the qt
                            # activation at kc==5, so ACT finishes qt(tt+1)
                            # before this tile's last exps and the next tile's
                            # scores start without waiting on ACT.
                            # (qp shares the "aux" psum slot with epi's bt
                            # tiles, so it must allocate after epi2's bt)
                            e0 = (kc - 2) * NEC // 4
                            e1 = (kc - 1) * NEC // 4
                            for ec in range(e0, e1):
                                emit_qproj_mm(tt + 1, ec, ps1, "aux")
                            if kc == 5:
                                emit_qproj_act(tt + 1)
                        if kc >= 1:
                            kp = kc - 1
                            gc = b * NKT + kp
                            emp = state[("emt", kp)]
                            st, sp = (kp == 0), (kp == NKT - 1)
                            nc.tensor.matmul(
                                cx0, v_sb[:, gc, 0:65], emp[:, 0, :],
                                start=st, stop=sp,
                            )
                            nc.tensor.matmul(
                                cx1, v_sb[:, gc, 65:130], emp[:, 1, :],
                                start=st, stop=sp,
                            )
                    emit_epi0(tt)
                    if tt > 0 and (tt - 1) % 2 == 1:
                        emit_outdma_pair((tt - 1) // 2)

                # final tile: its omms borrow the (now idle) sct psum slots so
                # the matmul->evac chain double-buffers through the tail
                # final tile: omms borrow the idle sct psum slots and the
                # evacs alternate DVE/ACT so the tail double-buffers
                tt = NTT - 1
                emit_epi1(tt)
                emit_epi2(tt)
                for oc in range(NOC):
                    emit_omm(tt, oc, ps2, "sct", act=(oc % 2 == 1))
                emit_outdma_pair((NTT - 1) // 2)

    nc.compile()
    _NC_CACHE[key] = nc
    return nc


def _pick_kc(mask):
    mx = max(int((mask[b] != 0).sum()) for b in range(B))
    for kc in (1024, 1280, 1536, 1792, 2048):
        if mx <= kc:
            return kc
    return 2048


def _weight_img(wT):
    # SBUF image of wX_sb[p, ec, j] = wT[ec*128+p, j], wT shape [E, JC]
    wT = np.ascontiguousarray(np.asarray(wT, np.float32))
    return np.ascontiguousarray(
        wT.reshape(NEC, 128, JC).transpose(1, 0, 2).reshape(128, NEC * JC)
    ).astype(BF)


def make_in_maps(query, key_value, mask, Wq, bq, Wk, bk, Wv, bv, Wo, bo, KC=1024):
    NKT = KC // 128
    xqT = np.ascontiguousarray(
        np.asarray(query, np.float32).reshape(T, E).T
    ).astype(BF)
    kvp = np.zeros((B, KC, E), np.float32)
    mbias = np.full((B, KC), -1.0e5, np.float32)
    kv = np.asarray(key_value, np.float32)
    for b in range(B):
        idx = np.nonzero(np.asarray(mask)[b] != 0)[0]
        n = min(len(idx), KC)
        kvp[b, :n] = kv[b][idx[:n]]
        mbias[b, :n] = 0.0
    xkT = np.ascontiguousarray(kvp.reshape(B * KC, E).T).astype(BF)
    in_maps = []
    for c in range(N_CORES):
        sl = slice(c * JC, (c + 1) * JC)
        aux = np.zeros((128, 3 + B * NKT), np.float32)
        aux[:, 0] = np.asarray(bq, np.float32)[sl]
        aux[:, 1] = np.asarray(bk, np.float32)[sl]
        aux[:, 2] = np.asarray(bv, np.float32)[sl]
        aux[:, 3:] = (
            mbias.reshape(B, NKT, 128).transpose(2, 0, 1).reshape(128, B * NKT)
        )
        wk_img = _weight_img(np.asarray(Wk)[sl, :].T)
        wv_img = _weight_img(np.asarray(Wv)[sl, :].T)
        wq_img = _weight_img(np.asarray(Wq)[sl, :].T)
        wo_img = np.ascontiguousarray(
            np.asarray(Wo, np.float32)[:, sl].T
        ).astype(BF)
        in_maps.append(
            {
                "xqT": xqT,
                "xkT": xkT,
                "wkvi": np.ascontiguousarray(
                    np.concatenate([wk_img, wv_img], axis=1)
                ),
                "wqoi": np.ascontiguousarray(
                    np.concatenate([wq_img, wo_img], axis=1)
                ),
                "auxi": aux,
            }
        )
    return in_maps


def kernel(query, key_value, mask, Wq, bq, Wk, bk, Wv, bv, Wo, bo):
    KC = _pick_kc(np.asarray(mask))
    nc = build(None, KC)
    in_maps = make_in_maps(
        query, key_value, mask, Wq, bq, Wk, bk, Wv, bv, Wo, bo, KC=KC
    )
    res = run_bass_kernel_spmd(nc, in_maps, list(range(N_CORES)))
    acc = np.zeros((E, T), np.float32)
    for c in range(N_CORES):
        acc += np.asarray(res.results[c]["outT"], dtype=np.float32)
    acc += np.asarray(bo, np.float32).reshape(E, 1)
    return np.ascontiguousarray(acc.T).reshape(B, LQ, E).astype(np.float32)
